# revision 11
# baseline (speedup 1.0000x reference)
"""Trainium2 Bass kernel for nn_Attention (B=32, P=577, D=768, 12 heads).

Strategy: data-parallel over batch — 4 batch elements per core on 8 cores,
zero collectives. The whole dataflow is kept *transposed* ([feature, token]
layouts) so every matmul consumes the previous stage's output directly with
no on-chip transposes:

  xT [768,2308] --(wqkvT)--> qkT [1536,577/batch] (e on partitions)
                        \--> V   [577/batch,768]  (token on partitions)
  per (b,h): ST = K Q^T  (ST[p,q], K=64, head pairs packed into PE row halves)
             AT = exp(scale*ST)            (ScalarE, softmax w/o max-subtract:
                                            |scale*S| < ~2, exp is safe)
  O_u^T[65,q] = [V|1]^T AT  (ones column gives softmax denominators in row 64)
  O^T = O_u^T * (1/denom broadcast)        (DVE + gpsimd partition_broadcast)
  yT [768,577/batch] = w_outT^T O^T + b_eff (v-bias folded into b_eff on host)

Scheduling: the PE sequencer executes in emission order, so the emitter
software-pipelines at sub-batch granularity. Per (b, j2) iteration the five
ST score units (each gated by the previous exp's PSUM drain) are spaced by
always-ready filler matmuls — next batch's projections, previous heads' AV,
previous batch's output projection:

  ST(qt0) PROJ ST(qt1) PROJ ST(qt2) AV ST(qt3) AV ST(qt4) OUT [PROJV]

which keeps PE busy while ACT (exp) and DVE (bias/norm) drain behind it.
V-tile PSUM->SBUF copies run on ACT; softmax normalization uses DVE
reciprocal + gpsimd partition_broadcast + DVE multiply. The final output is
stored bf16 to halve output DMA traffic (converted back to f32 on host).
Inputs are cast to bf16 on host; accumulation is f32 in PSUM.
"""

import numpy as np
import ml_dtypes

import concourse.bass as bass
import concourse.tile as tile
from concourse import bacc, mybir
from concourse.bass_utils import run_bass_kernel_spmd

# problem dims (hardcoded per harness contract)
B, PL, D = 32, 577, 768
H, S = 12, 64
NCORES = 8
NB = B // NCORES          # 4 batches per core
T = NB * PL               # 2308 tokens per core
P = 128
DT = D // P               # 6 contraction tiles
SCALE = float((D // 8) ** -0.5)   # 96**-0.5 (module bug kept faithful)

FB = mybir.dt.bfloat16
F32 = mybir.dt.float32

QS = [128, 128, 128, 128, 65]          # q-subtiles of 577
PCH = [(0, 512), (512, 65)]            # p-chunks of 577
VCH = [(0, 512), (512, 256)]           # V projection chunks of 768


def build_bass(reps=1):
    nc = bacc.Bacc("TRN2", target_bir_lowering=False, debug=False,
                   num_devices=NCORES)

    x_t = nc.dram_tensor("x_t", [D, T], FB, kind="ExternalInput").ap()
    w_qkv_t = nc.dram_tensor("w_qkv_t", [D, 3 * D], FB, kind="ExternalInput").ap()
    w_out_t = nc.dram_tensor("w_out_t", [D, D], FB, kind="ExternalInput").ap()
    b_qk = nc.dram_tensor("b_qk", [P, 12], F32, kind="ExternalInput").ap()
    b_out = nc.dram_tensor("b_out", [P, DT], F32, kind="ExternalInput").ap()
    out_d = nc.dram_tensor("out", [D, T], FB, kind="ExternalOutput").ap()
    out_v = out_d.rearrange("(o p) t -> p o t", p=P)
    xv = x_t.rearrange("(o p) t -> p o t", p=P)
    wv = w_qkv_t.rearrange("(o p) e -> p o e", p=P)

    with tile.TileContext(nc) as tc:
      for _rep in range(reps):  # >1 only for differential benchmarking
        with tc.tile_pool(name="singles", bufs=1) as singles, \
             tc.tile_pool(name="bt", bufs=2) as btp, \
             tc.tile_pool(name="atp", bufs=6) as atpool, \
             tc.tile_pool(name="nrm", bufs=6) as nrm, \
             tc.tile_pool(name="yout", bufs=4) as ypool, \
             tc.tile_pool(name="pst", bufs=2, space="PSUM") as pst, \
             tc.tile_pool(name="pg", bufs=2, space="PSUM") as pg:
            # per-k input tiles so matmuls start after the first k arrives;
            # x DMA split per batch so batch-0 projections gate only on the
            # first quarter of each k-tile
            # biases on the ACT DMA queue (tiny; gate the DVE bias-adds of
            # every proj unit) so they don't delay the first weight DMA on SP
            bqk = singles.tile([P, 12], F32, tag="bqk")
            nc.scalar.dma_start(bqk[:], b_qk)
            bo = singles.tile([P, DT], F32, tag="bo")
            nc.scalar.dma_start(bo[:], b_out)
            # inputs split across both HWDGE queues (SP + ACT) so the two
            # rings stream in parallel; q/k weight halves + batch-0 x first
            xt, wqkv = [], []
            for k in range(DT):
                wk = singles.tile([P, 3 * D], FB, tag=f"wq{k}", name=f"wq{k}")
                nc.sync.dma_start(wk[:, D:3 * D], wv[:, k, D:3 * D])
                nc.scalar.dma_start(wk[:, 0:D], wv[:, k, 0:D])
                wqkv.append(wk)
                xk = singles.tile([P, T], FB, tag=f"xt{k}", name=f"xt{k}")
                nc.scalar.dma_start(xk[:, 0:PL], xv[:, k, 0:PL])
                xt.append(xk)
            # out-proj weights: one plain 2D DMA per 128-row block (a single
            # 3D-AP DMA here hits the slow SWDGE descriptor path and parks
            # the SP sequencer for ~50us)
            wo = singles.tile([P, DT, D], FB, tag="wo")
            wov = w_out_t.rearrange("(o p) e -> o p e", p=P)
            for o in range(DT):
                (nc.sync if o % 2 == 0 else nc.scalar).dma_start(
                    wo[:, o, :], wov[o])
            for k in range(DT):
                for b in range(1, NB):
                    (nc.sync if (k + b) % 2 == 0 else nc.scalar).dma_start(
                        xt[k][:, b * PL:(b + 1) * PL],
                        xv[:, k, b * PL:(b + 1) * PL])

            qkt, vbuf, ot, at = {}, {}, {}, {}

            def emit_proj_j(b, j):
                # q/k projection for one 128-feature block -> qkT[(b,j)]
                ps = pg.tile([P, D], F32, tag="prj", name="psqk")
                for k in range(DT):
                    for (c0, cw) in PCH:
                        nc.tensor.matmul(
                            ps[:, c0:c0 + cw],
                            lhsT=wqkv[k][:, D + j * P: D + (j + 1) * P],
                            rhs=xt[k][:, b * PL + c0: b * PL + c0 + cw],
                            start=(k == 0), stop=(k == DT - 1),
                            skip_group_check=True)
                qt_tile = btp.tile([P, PL], FB, tag=f"qkt{j}", name=f"qkt{j}")
                nc.vector.tensor_scalar_add(qt_tile[:], ps[:, 0:PL],
                                            bqk[:, j:j + 1])
                qkt[(b, j)] = qt_tile

            def emit_projv(b, tt):
                # V projection for one 128-token block -> vbuf[(b,tt)]
                rows = QS[tt]
                ps = pg.tile([P, D], F32, tag="prj", name="psv")
                for k in range(DT):
                    for (c0, cw) in VCH:
                        nc.tensor.matmul(
                            ps[:rows, c0:c0 + cw],
                            lhsT=xt[k][:, b * PL + tt * P: b * PL + tt * P + rows],
                            rhs=wqkv[k][:, c0:c0 + cw],
                            start=(k == 0), stop=(k == DT - 1),
                            skip_group_check=True)
                vt = btp.tile([P, H, S + 1], FB, tag=f"v{tt}", name=f"v{tt}")
                nc.gpsimd.memset(vt[:, :, S:S + 1], 1.0)
                nc.scalar.copy(
                    vt[:rows, :, 0:S],
                    ps[:rows].rearrange("p (h s) -> p h s", h=H))
                vbuf[(b, tt)] = vt

            def emit_st(b, j2, qt):
                # scores + exp for one 128-key-token block of a head pair
                rows = QS[qt]
                qs_ = qkt[(b, j2)]
                ks_ = qkt[(b, 6 + j2)]
                if qt == 0:
                    at[(b, j2, 0)] = atpool.tile([P, 5, PL], FB, tag="at",
                                                 name="at0")
                    at[(b, j2, 1)] = atpool.tile([P, 5, PL], FB, tag="at",
                                                 name="at1")
                st0 = pst.tile([P, PL], F32, tag="st", name="st0")
                st1 = pst.tile([P, PL], F32, tag="st", name="st1")
                for (c0, cw) in PCH:
                    nc.tensor.matmul(
                        st0[:rows, c0:c0 + cw],
                        lhsT=ks_[0:64, qt * P: qt * P + rows],
                        rhs=qs_[0:64, c0:c0 + cw],
                        start=True, stop=True,
                        skip_group_check=True)
                    nc.tensor.matmul(
                        st1[:rows, c0:c0 + cw],
                        lhsT=ks_[64:128, qt * P: qt * P + rows],
                        rhs=qs_[64:128, c0:c0 + cw],
                        start=True, stop=True,
                        skip_group_check=True)
                nc.scalar.activation(
                    at[(b, j2, 0)][:rows, qt, :], st0[:rows, :],
                    mybir.ActivationFunctionType.Exp, scale=SCALE)
                nc.scalar.activation(
                    at[(b, j2, 1)][:rows, qt, :], st1[:rows, :],
                    mybir.ActivationFunctionType.Exp, scale=SCALE)

            avs_t, recb_t = {}, {}

            def emit_av_front(b, h):
                # attention-weighted V matmuls + fast PSUM drain (copy to
                # SBUF) + reciprocal; the normalize-multiply is deferred one
                # slot (emit_av_norm) so the PSUM buffer frees after just the
                # copy instead of the whole recip->broadcast->mul chain
                j2, half = h // 2, h % 2
                if h == 0:
                    for j in range(DT):
                        ot[(b, j)] = btp.tile([P, PL], FB, tag=f"ot{j}",
                                              name=f"ot{j}")
                ath = at[(b, j2, half)]
                av = pg.tile([P, D], F32, tag="prj", name="av")
                for qt in range(5):
                    rows = QS[qt]
                    for (c0, cw) in PCH:
                        nc.tensor.matmul(
                            av[0:S + 1, c0:c0 + cw],
                            lhsT=vbuf[(b, qt)][:rows, h, :],
                            rhs=ath[:rows, qt, c0:c0 + cw],
                            start=(qt == 0), stop=(qt == 4),
                            skip_group_check=True)
                avs = nrm.tile([S + 1, PL], F32, tag="avs", name="avs")
                nc.vector.tensor_copy(avs[:], av[0:S + 1, 0:PL])
                rec = nrm.tile([1, PL], F32, tag="rec", name="rec")
                nc.vector.reciprocal(rec[:], avs[S:S + 1, :])
                recb = nrm.tile([64, PL], F32, tag="recb", name="recb")
                nc.gpsimd.partition_broadcast(recb[:], rec[:])
                avs_t[(b, h)] = avs
                recb_t[(b, h)] = recb

            def emit_av_norm(b, h):
                j2, hp = h // 2, (h % 2) * 64
                avs = avs_t.pop((b, h))
                recb = recb_t.pop((b, h))
                if hp == 0:
                    nc.vector.tensor_mul(
                        out=ot[(b, j2)][0:S, :], in0=avs[0:S, :], in1=recb[:])
                else:
                    tmp = nrm.tile([64, PL], FB, tag="tmp", name="tmp")
                    nc.vector.tensor_mul(out=tmp[:], in0=avs[0:S, :],
                                         in1=recb[:])
                    # gpsimd SWDGE queue: keeps this partition-shift copy off
                    # the SP queue where it would sit behind output DMAs
                    nc.gpsimd.dma_start(ot[(b, j2)][64:128, :], tmp[:])

            def emit_out(b, m):
                # output projection for one 128-feature block + store
                ps = pg.tile([P, D], F32, tag="prj", name="psy")
                for k in range(DT):
                    for (c0, cw) in PCH:
                        nc.tensor.matmul(
                            ps[:, c0:c0 + cw],
                            lhsT=wo[:, k, m * P:(m + 1) * P],
                            rhs=ot[(b, k)][:, c0:c0 + cw],
                            start=(k == 0), stop=(k == DT - 1),
                            skip_group_check=True)
                ysb = ypool.tile([P, PL], FB, tag="ysb", name="ysb")
                nc.vector.tensor_scalar_add(ysb[:], ps[:, 0:PL], bo[:, m:m + 1])
                (nc.sync if m % 2 == 0 else nc.scalar).dma_start(
                    out_v[:, m, b * PL:(b + 1) * PL], ysb[:])

            # ---- software-pipelined emission schedule ----
            # Each iteration (b, j2) emits its own batch's NEXT j2-pair of
            # q/k projections, so every batch (including the last) carries
            # its projection filler; batch b+1's first pair rides iter
            # (b, 5). Prologue covers only batch 0's first pair + V.
            emit_proj_j(0, 0)
            emit_proj_j(0, 6)
            for tt in range(5):
                emit_projv(0, tt)

            def proj_filler(b, j2, which):
                # which=0 -> first PROJ slot, which=1 -> second
                if j2 < 5:
                    j = (j2 + 1) if which == 0 else (j2 + 7)
                    emit_proj_j(b, j)
                elif b + 1 < NB:
                    emit_proj_j(b + 1, 0 if which == 0 else 6)

            for b in range(NB):
                for j2 in range(6):
                    # previous head pair to normalize: (b, j2-1), wrapping to
                    # (b-1, 5) at j2=0
                    pb, pj2 = (b, j2 - 1) if j2 > 0 else (b - 1, 5)
                    emit_st(b, j2, 0)
                    # PROJV early so its ACT V-copy's psum wait never
                    # head-of-line blocks the exp stream behind it
                    if b + 1 < NB and j2 < 5:
                        emit_projv(b + 1, j2)
                    emit_st(b, j2, 1)
                    proj_filler(b, j2, 0)
                    emit_st(b, j2, 2)
                    if pb >= 0:
                        emit_av_front(pb, 2 * pj2)
                    emit_st(b, j2, 3)
                    if pb >= 0:
                        emit_av_front(pb, 2 * pj2 + 1)
                        emit_av_norm(pb, 2 * pj2)
                    emit_st(b, j2, 4)
                    if pb >= 0:
                        emit_av_norm(pb, 2 * pj2 + 1)
                    if b > 0:
                        emit_out(b - 1, j2)
                    proj_filler(b, j2, 1)
            # epilogue: last head pair + last batch's output projection
            emit_av_front(NB - 1, 10)
            emit_av_front(NB - 1, 11)
            emit_av_norm(NB - 1, 10)
            emit_av_norm(NB - 1, 11)
            for m in range(DT):
                emit_out(NB - 1, m)

    nc.compile()
    return nc


_NC = None


def _get_nc():
    global _NC
    if _NC is None:
        _NC = build_bass()
    return _NC


def make_in_maps(x, qkv_w, qkv_b, out_w, out_b):
    """Host-side shard + layout prep. Returns per-core input dicts."""
    bf16 = ml_dtypes.bfloat16
    x = np.asarray(x, dtype=np.float32)
    qkv_w = np.asarray(qkv_w, dtype=np.float32)
    qkv_b = np.asarray(qkv_b, dtype=np.float32)
    out_w = np.asarray(out_w, dtype=np.float32)
    out_b = np.asarray(out_b, dtype=np.float32)

    w_qkv_t = np.ascontiguousarray(qkv_w.T).astype(bf16)          # [768, 2304]
    w_out_t = np.ascontiguousarray(out_w.T).astype(bf16)          # [768, 768]
    b_qk = np.ascontiguousarray(qkv_b[D:3 * D].reshape(12, P).T)  # [128, 12]
    # v-bias passes linearly through the output projection (softmax rows sum
    # to 1): fold it into an effective output bias.
    b_eff = out_b + out_w @ qkv_b[0:D]
    b_out = np.ascontiguousarray(b_eff.reshape(DT, P).T)          # [128, 6]

    in_maps = []
    for c in range(NCORES):
        xc = x[c * NB:(c + 1) * NB].reshape(T, D)                 # [2308, 768]
        x_t = np.ascontiguousarray(xc.T).astype(bf16)             # [768, 2308]
        in_maps.append({
            "x_t": x_t,
            "w_qkv_t": w_qkv_t,
            "w_out_t": w_out_t,
            "b_qk": b_qk.astype(np.float32),
            "b_out": b_out.astype(np.float32),
        })
    return in_maps


def assemble_output(results):
    """Per-core 'out' [768, 2308] bf16 -> full [32, 577, 768] f32."""
    y = np.empty((B, PL, D), dtype=np.float32)
    for c in range(NCORES):
        yt = results[c]["out"].astype(np.float32)                 # [768, 2308]
        y[c * NB:(c + 1) * NB] = yt.T.reshape(NB, PL, D)
    return y


def run(x, qkv_w, qkv_b, out_w, out_b, trace=False):
    nc = _get_nc()
    in_maps = make_in_maps(x, qkv_w, qkv_b, out_w, out_b)
    res = run_bass_kernel_spmd(nc, in_maps, core_ids=list(range(NCORES)),
                               trace=trace)
    return assemble_output(res.results), res


def kernel(x, qkv_w, qkv_b, out_w, out_b):
    y, _ = run(x, qkv_w, qkv_b, out_w, out_b)
    return y


# revision 24
# speedup vs baseline: 1.1800x; 1.1800x over previous
"""Trainium2 Bass kernel for nn_Attention (B=32, P=577, D=768, 12 heads).

Strategy: data-parallel over batch — 4 batch elements per core on 8 cores,
zero collectives. The whole dataflow is kept *transposed* ([feature, token]
layouts) so every matmul consumes the previous stage's output directly with
no on-chip transposes:

  xT [768,2308] --(wqkvT)--> qkT [1536,577/batch] (e on partitions)
                        \--> V   [577/batch,768]  (token on partitions)
  per (b,h): ST = K Q^T  (ST[p,q], K=64, head pairs packed into PE row halves)
             AT = exp(scale*ST)            (ScalarE, softmax w/o max-subtract:
                                            |scale*S| < ~2, exp is safe)
  O_u^T[65,q] = [V|1]^T AT  (ones column gives softmax denominators in row 64)
  O^T = O_u^T * (1/denom broadcast)        (DVE + gpsimd partition_broadcast)
  yT [768,577/batch] = w_outT^T O^T + b_eff (v-bias folded into b_eff on host)

Scheduling: the PE sequencer executes in emission order, so the emitter
software-pipelines at sub-batch granularity. Per (b, j2) iteration the five
ST score units (each gated by the previous exp's PSUM drain) are spaced by
always-ready filler matmuls — the batch's own next q/k projection pair,
next batch's V projection, previous heads' AV, previous batch's output
projection:

  ST(qt0) PROJV ST(qt1) PROJ ST(qt2) AVF ST(qt3) AVF+AVN ST(qt4) AVN OUT PROJ

which keeps PE busy while ACT (exp) and DVE (bias/copy/norm) drain behind
it. Every batch (including the last) carries its own projection filler, so
no iteration is starved. AV PSUM is drained by a single fast DVE copy to
SBUF (AVF); the reciprocal/broadcast/multiply chain (AVN) runs one slot
later off the copy, so the PSUM buffer recycles ~2us sooner. The odd-head
partition-shift copies ride the gpsimd SWDGE queue, away from the SP output
queue. The final output is stored bf16 to halve output DMA traffic
(converted back to f32 on host). Inputs are cast to bf16 on host;
accumulation is f32 in PSUM.

Measured (axon trn2, differential reps=10 block-median): ~390-415us vs
~465-495us for the coarse-interleave baseline; TimelineSim predicts 306us
(PE busy 263us/86%), the residual being unmodeled per-instruction HW
overhead concentrated on ACT/DVE.
"""

import numpy as np
import ml_dtypes

import concourse.bass as bass
import concourse.tile as tile
from concourse import bacc, mybir
from concourse.bass_utils import run_bass_kernel_spmd

# problem dims (hardcoded per harness contract)
B, PL, D = 32, 577, 768
H, S = 12, 64
NCORES = 8
NB = B // NCORES          # 4 batches per core
T = NB * PL               # 2308 tokens per core
P = 128
DT = D // P               # 6 contraction tiles
SCALE = float((D // 8) ** -0.5)   # 96**-0.5 (module bug kept faithful)

FB = mybir.dt.bfloat16
F32 = mybir.dt.float32

QS = [128, 128, 128, 128, 65]          # q-subtiles of 577
PCH = [(0, 512), (512, 65)]            # p-chunks of 577
VCH = [(0, 512), (512, 256)]           # V projection chunks of 768


def build_bass(reps=1, in_q="mix", out_q="sp", shared_pools=False,
               bias_eng="dve", vcopy_eng="dve"):
    nc = bacc.Bacc("TRN2", target_bir_lowering=False, debug=False,
                   num_devices=NCORES)

    x_t = nc.dram_tensor("x_t", [D, T], FB, kind="ExternalInput").ap()
    w_qkv_t = nc.dram_tensor("w_qkv_t", [D, 3 * D], FB, kind="ExternalInput").ap()
    w_out_t = nc.dram_tensor("w_out_t", [D, D], FB, kind="ExternalInput").ap()
    b_qk = nc.dram_tensor("b_qk", [P, 12], F32, kind="ExternalInput").ap()
    b_out = nc.dram_tensor("b_out", [P, DT], F32, kind="ExternalInput").ap()
    out_d = nc.dram_tensor("out", [D, T], FB, kind="ExternalOutput").ap()
    out_v = out_d.rearrange("(o p) t -> p o t", p=P)
    xv = x_t.rearrange("(o p) t -> p o t", p=P)
    wv = w_qkv_t.rearrange("(o p) e -> p o e", p=P)

    with tile.TileContext(nc) as tc:
      def emit_rep(singles, btp, atpool, nrm, ypool, pst, pg):
            # per-k input tiles so matmuls start after the first k arrives;
            # x DMA split per batch so batch-0 projections gate only on the
            # first quarter of each k-tile
            # biases on the ACT DMA queue (tiny; gate the DVE bias-adds of
            # every proj unit) so they don't delay the first weight DMA on SP
            bqk = singles.tile([P, 12], F32, tag="bqk")
            nc.scalar.dma_start(bqk[:], b_qk)
            bo = singles.tile([P, DT], F32, tag="bo")
            nc.scalar.dma_start(bo[:], b_out)
            # inputs optionally split across both HWDGE queues (SP + ACT) so
            # the two rings stream in parallel; q/k halves + batch-0 x first
            q2 = nc.scalar if in_q == "mix" else nc.sync
            xt, wqkv = [], []
            for k in range(DT):
                wk = singles.tile([P, 3 * D], FB, tag=f"wq{k}", name=f"wq{k}")
                nc.sync.dma_start(wk[:, D:3 * D], wv[:, k, D:3 * D])
                q2.dma_start(wk[:, 0:D], wv[:, k, 0:D])
                wqkv.append(wk)
                xk = singles.tile([P, T], FB, tag=f"xt{k}", name=f"xt{k}")
                q2.dma_start(xk[:, 0:PL], xv[:, k, 0:PL])
                xt.append(xk)
            # out-proj weights: one plain 2D DMA per 128-row block (a single
            # 3D-AP DMA here hits the slow SWDGE descriptor path and parks
            # the SP sequencer for ~50us)
            wo = singles.tile([P, DT, D], FB, tag="wo")
            wov = w_out_t.rearrange("(o p) e -> o p e", p=P)
            for o in range(DT):
                (nc.sync if o % 2 == 0 else q2).dma_start(
                    wo[:, o, :], wov[o])
            for k in range(DT):
                for b in range(1, NB):
                    (nc.sync if (k + b) % 2 == 0 else q2).dma_start(
                        xt[k][:, b * PL:(b + 1) * PL],
                        xv[:, k, b * PL:(b + 1) * PL])

            qkt, vbuf, ot, at = {}, {}, {}, {}

            def emit_proj_j(b, j):
                # q/k projection for one 128-feature block -> qkT[(b,j)]
                ps = pg.tile([P, D], F32, tag="prj", name="psqk")
                for k in range(DT):
                    for (c0, cw) in PCH:
                        nc.tensor.matmul(
                            ps[:, c0:c0 + cw],
                            lhsT=wqkv[k][:, D + j * P: D + (j + 1) * P],
                            rhs=xt[k][:, b * PL + c0: b * PL + c0 + cw],
                            start=(k == 0), stop=(k == DT - 1),
                            skip_group_check=True)
                qt_tile = btp.tile([P, PL], FB, tag=f"qkt{j}", name=f"qkt{j}")
                beng = nc.gpsimd if bias_eng == "pool" else nc.vector
                beng.tensor_scalar_add(qt_tile[:], ps[:, 0:PL],
                                       bqk[:, j:j + 1])
                qkt[(b, j)] = qt_tile

            def emit_projv(b, tt):
                # V projection for one 128-token block -> vbuf[(b,tt)]
                rows = QS[tt]
                ps = pg.tile([P, D], F32, tag="prj", name="psv")
                for k in range(DT):
                    for (c0, cw) in VCH:
                        nc.tensor.matmul(
                            ps[:rows, c0:c0 + cw],
                            lhsT=xt[k][:, b * PL + tt * P: b * PL + tt * P + rows],
                            rhs=wqkv[k][:, c0:c0 + cw],
                            start=(k == 0), stop=(k == DT - 1),
                            skip_group_check=True)
                vt = btp.tile([P, H, S + 1], FB, tag=f"v{tt}", name=f"v{tt}")
                nc.gpsimd.memset(vt[:, :, S:S + 1], 1.0)
                if vcopy_eng == "act":
                    nc.scalar.copy(
                        vt[:rows, :, 0:S],
                        ps[:rows].rearrange("p (h s) -> p h s", h=H))
                else:
                    nc.vector.tensor_copy(
                        vt[:rows, :, 0:S],
                        ps[:rows].rearrange("p (h s) -> p h s", h=H))
                vbuf[(b, tt)] = vt

            def emit_st(b, j2, qt):
                # scores + exp for one 128-key-token block of a head pair
                rows = QS[qt]
                qs_ = qkt[(b, j2)]
                ks_ = qkt[(b, 6 + j2)]
                if qt == 0:
                    at[(b, j2, 0)] = atpool.tile([P, 5, PL], FB, tag="at",
                                                 name="at0")
                    at[(b, j2, 1)] = atpool.tile([P, 5, PL], FB, tag="at",
                                                 name="at1")
                st0 = pst.tile([P, PL], F32, tag="st", name="st0")
                st1 = pst.tile([P, PL], F32, tag="st", name="st1")
                for (c0, cw) in PCH:
                    nc.tensor.matmul(
                        st0[:rows, c0:c0 + cw],
                        lhsT=ks_[0:64, qt * P: qt * P + rows],
                        rhs=qs_[0:64, c0:c0 + cw],
                        start=True, stop=True,
                        skip_group_check=True)
                    nc.tensor.matmul(
                        st1[:rows, c0:c0 + cw],
                        lhsT=ks_[64:128, qt * P: qt * P + rows],
                        rhs=qs_[64:128, c0:c0 + cw],
                        start=True, stop=True,
                        skip_group_check=True)
                nc.scalar.activation(
                    at[(b, j2, 0)][:rows, qt, :], st0[:rows, :],
                    mybir.ActivationFunctionType.Exp, scale=SCALE)
                nc.scalar.activation(
                    at[(b, j2, 1)][:rows, qt, :], st1[:rows, :],
                    mybir.ActivationFunctionType.Exp, scale=SCALE)

            avs_t, recb_t = {}, {}

            def emit_av_front(b, h):
                # attention-weighted V matmuls + fast PSUM drain (copy to
                # SBUF) + reciprocal; the normalize-multiply is deferred one
                # slot (emit_av_norm) so the PSUM buffer frees after just the
                # copy instead of the whole recip->broadcast->mul chain
                j2, half = h // 2, h % 2
                if h == 0:
                    for j in range(DT):
                        ot[(b, j)] = btp.tile([P, PL], FB, tag=f"ot{j}",
                                              name=f"ot{j}")
                ath = at[(b, j2, half)]
                av = pg.tile([P, D], F32, tag="prj", name="av")
                for qt in range(5):
                    rows = QS[qt]
                    for (c0, cw) in PCH:
                        nc.tensor.matmul(
                            av[0:S + 1, c0:c0 + cw],
                            lhsT=vbuf[(b, qt)][:rows, h, :],
                            rhs=ath[:rows, qt, c0:c0 + cw],
                            start=(qt == 0), stop=(qt == 4),
                            skip_group_check=True)
                avs = nrm.tile([S + 1, PL], F32, tag="avs", name="avs")
                nc.vector.tensor_copy(avs[:], av[0:S + 1, 0:PL])
                rec = nrm.tile([1, PL], F32, tag="rec", name="rec")
                nc.vector.reciprocal(rec[:], avs[S:S + 1, :])
                recb = nrm.tile([64, PL], F32, tag="recb", name="recb")
                nc.gpsimd.partition_broadcast(recb[:], rec[:])
                avs_t[(b, h)] = avs
                recb_t[(b, h)] = recb

            def emit_av_norm(b, h):
                j2, hp = h // 2, (h % 2) * 64
                avs = avs_t.pop((b, h))
                recb = recb_t.pop((b, h))
                if hp == 0:
                    nc.vector.tensor_mul(
                        out=ot[(b, j2)][0:S, :], in0=avs[0:S, :], in1=recb[:])
                else:
                    tmp = nrm.tile([64, PL], FB, tag="tmp", name="tmp")
                    nc.vector.tensor_mul(out=tmp[:], in0=avs[0:S, :],
                                         in1=recb[:])
                    # gpsimd SWDGE queue: keeps this partition-shift copy off
                    # the SP queue where it would sit behind output DMAs
                    nc.gpsimd.dma_start(ot[(b, j2)][64:128, :], tmp[:])

            def emit_out(b, m):
                # output projection for one 128-feature block + store
                ps = pg.tile([P, D], F32, tag="prj", name="psy")
                for k in range(DT):
                    for (c0, cw) in PCH:
                        nc.tensor.matmul(
                            ps[:, c0:c0 + cw],
                            lhsT=wo[:, k, m * P:(m + 1) * P],
                            rhs=ot[(b, k)][:, c0:c0 + cw],
                            start=(k == 0), stop=(k == DT - 1),
                            skip_group_check=True)
                ysb = ypool.tile([P, PL], FB, tag="ysb", name="ysb")
                beng = nc.gpsimd if bias_eng == "pool" else nc.vector
                beng.tensor_scalar_add(ysb[:], ps[:, 0:PL], bo[:, m:m + 1])
                oq = nc.scalar if (out_q == "mix" and m % 2 == 1) else nc.sync
                oq.dma_start(out_v[:, m, b * PL:(b + 1) * PL], ysb[:])

            # ---- software-pipelined emission schedule ----
            # Each iteration (b, j2) emits its own batch's NEXT j2-pair of
            # q/k projections, so every batch (including the last) carries
            # its projection filler; batch b+1's first pair rides iter
            # (b, 5). Prologue covers only batch 0's first pair + V.
            emit_proj_j(0, 0)
            emit_proj_j(0, 6)
            for tt in range(5):
                emit_projv(0, tt)

            def proj_filler(b, j2, which):
                # which=0 -> first PROJ slot, which=1 -> second
                if j2 < 5:
                    j = (j2 + 1) if which == 0 else (j2 + 7)
                    emit_proj_j(b, j)
                elif b + 1 < NB:
                    emit_proj_j(b + 1, 0 if which == 0 else 6)

            for b in range(NB):
                for j2 in range(6):
                    # previous head pair to normalize: (b, j2-1), wrapping to
                    # (b-1, 5) at j2=0
                    pb, pj2 = (b, j2 - 1) if j2 > 0 else (b - 1, 5)
                    emit_st(b, j2, 0)
                    # PROJV early so its ACT V-copy's psum wait never
                    # head-of-line blocks the exp stream behind it
                    if b + 1 < NB and j2 < 5:
                        emit_projv(b + 1, j2)
                    emit_st(b, j2, 1)
                    proj_filler(b, j2, 0)
                    emit_st(b, j2, 2)
                    if pb >= 0:
                        emit_av_front(pb, 2 * pj2)
                    emit_st(b, j2, 3)
                    if pb >= 0:
                        emit_av_front(pb, 2 * pj2 + 1)
                        emit_av_norm(pb, 2 * pj2)
                    emit_st(b, j2, 4)
                    if pb >= 0:
                        emit_av_norm(pb, 2 * pj2 + 1)
                    if b > 0:
                        emit_out(b - 1, j2)
                    proj_filler(b, j2, 1)
            # epilogue: last head pair + last batch's output projection
            emit_av_front(NB - 1, 10)
            emit_av_front(NB - 1, 11)
            emit_av_norm(NB - 1, 10)
            emit_av_norm(NB - 1, 11)
            for m in range(DT):
                emit_out(NB - 1, m)

      def open_pools():
        return (tc.tile_pool(name="singles", bufs=1),
                tc.tile_pool(name="bt", bufs=2),
                tc.tile_pool(name="atp", bufs=6),
                tc.tile_pool(name="nrm", bufs=6),
                tc.tile_pool(name="yout", bufs=4),
                tc.tile_pool(name="pst", bufs=2, space="PSUM"),
                tc.tile_pool(name="pg", bufs=2, space="PSUM"))

      import contextlib
      if shared_pools:
          # one pool generation across reps: no inter-rep drain barrier
          with contextlib.ExitStack() as st:
              pools = [st.enter_context(p) for p in open_pools()]
              for _rep in range(reps):
                  emit_rep(*pools)
      else:
          for _rep in range(reps):
              with contextlib.ExitStack() as st:
                  pools = [st.enter_context(p) for p in open_pools()]
                  emit_rep(*pools)

    nc.compile()
    return nc


_NC = None


def _get_nc():
    global _NC
    if _NC is None:
        _NC = build_bass()
    return _NC


def make_in_maps(x, qkv_w, qkv_b, out_w, out_b):
    """Host-side shard + layout prep. Returns per-core input dicts."""
    bf16 = ml_dtypes.bfloat16
    x = np.asarray(x, dtype=np.float32)
    qkv_w = np.asarray(qkv_w, dtype=np.float32)
    qkv_b = np.asarray(qkv_b, dtype=np.float32)
    out_w = np.asarray(out_w, dtype=np.float32)
    out_b = np.asarray(out_b, dtype=np.float32)

    w_qkv_t = np.ascontiguousarray(qkv_w.T).astype(bf16)          # [768, 2304]
    w_out_t = np.ascontiguousarray(out_w.T).astype(bf16)          # [768, 768]
    b_qk = np.ascontiguousarray(qkv_b[D:3 * D].reshape(12, P).T)  # [128, 12]
    # v-bias passes linearly through the output projection (softmax rows sum
    # to 1): fold it into an effective output bias.
    b_eff = out_b + out_w @ qkv_b[0:D]
    b_out = np.ascontiguousarray(b_eff.reshape(DT, P).T)          # [128, 6]

    in_maps = []
    for c in range(NCORES):
        xc = x[c * NB:(c + 1) * NB].reshape(T, D)                 # [2308, 768]
        x_t = np.ascontiguousarray(xc.T).astype(bf16)             # [768, 2308]
        in_maps.append({
            "x_t": x_t,
            "w_qkv_t": w_qkv_t,
            "w_out_t": w_out_t,
            "b_qk": b_qk.astype(np.float32),
            "b_out": b_out.astype(np.float32),
        })
    return in_maps


def assemble_output(results):
    """Per-core 'out' [768, 2308] bf16 -> full [32, 577, 768] f32."""
    y = np.empty((B, PL, D), dtype=np.float32)
    for c in range(NCORES):
        yt = results[c]["out"].astype(np.float32)                 # [768, 2308]
        y[c * NB:(c + 1) * NB] = yt.T.reshape(NB, PL, D)
    return y


def run(x, qkv_w, qkv_b, out_w, out_b, trace=False):
    nc = _get_nc()
    in_maps = make_in_maps(x, qkv_w, qkv_b, out_w, out_b)
    res = run_bass_kernel_spmd(nc, in_maps, core_ids=list(range(NCORES)),
                               trace=trace)
    return assemble_output(res.results), res


def kernel(x, qkv_w, qkv_b, out_w, out_b):
    y, _ = run(x, qkv_w, qkv_b, out_w, out_b)
    return y


# revision 25
# speedup vs baseline: 1.7641x; 1.4950x over previous
"""Trainium2 Bass kernel for nn_Attention (B=32, P=577, D=768, 12 heads).

Strategy: data-parallel over batch — 4 batch elements per core on 8 cores,
zero collectives. The whole dataflow is kept *transposed* ([feature, token]
layouts) so every matmul consumes the previous stage's output directly with
no on-chip transposes:

  xT [768,2308] --(wqkvT)--> qkT [1536,577/batch] (e on partitions)
                        \--> V   [577/batch,768]  (token on partitions)
  per (b,h): ST = K Q^T  (ST[p,q], K=64, head pairs packed into PE row halves)
             AT = exp(scale*ST)            (ScalarE, softmax w/o max-subtract:
                                            |scale*S| < ~2, exp is safe)
  O_u^T[65,q] = [V|1]^T AT  (ones column gives softmax denominators in row 64)
  O^T = O_u^T * (1/denom broadcast)        (DVE + gpsimd partition_broadcast)
  yT [768,577/batch] = w_outT^T O^T + b_eff (v-bias folded into b_eff on host)

Scheduling: the PE sequencer executes in emission order, so the emitter
software-pipelines at sub-batch granularity. Per (b, j2) iteration the five
ST score units (each gated by the previous exp's PSUM drain) are spaced by
always-ready filler matmuls — the batch's own next q/k projection pair,
next batch's V projection, previous heads' AV, previous batch's output
projection:

  ST(qt0) PROJV ST(qt1) PROJ ST(qt2) AVF ST(qt3) AVF+AVN ST(qt4) AVN OUT PROJ

which keeps PE busy while ACT (exp) and DVE (bias/copy/norm) drain behind
it. Every batch (including the last) carries its own projection filler, so
no iteration is starved. AV PSUM is drained by a single fast DVE copy to
SBUF (AVF); the reciprocal/broadcast/multiply chain (AVN) runs one slot
later off the copy, so the PSUM buffer recycles ~2us sooner. The odd-head
partition-shift copies ride the gpsimd SWDGE queue, away from the SP output
queue. The final output is stored bf16 to halve output DMA traffic
(converted back to f32 on host). Inputs are cast to bf16 on host;
accumulation is f32 in PSUM.

Measured (axon trn2, differential reps=10 block-median): ~390-415us vs
~465-495us for the coarse-interleave baseline; TimelineSim predicts 306us
(PE busy 263us/86%), the residual being unmodeled per-instruction HW
overhead concentrated on ACT/DVE.
"""

import numpy as np
import ml_dtypes

import concourse.bass as bass
import concourse.tile as tile
from concourse import bacc, mybir
from concourse.bass_utils import run_bass_kernel_spmd

# problem dims (hardcoded per harness contract)
B, PL, D = 32, 577, 768
H, S = 12, 64
NCORES = 8
NB = B // NCORES          # 4 batches per core
T = NB * PL               # 2308 tokens per core
P = 128
DT = D // P               # 6 contraction tiles
SCALE = float((D // 8) ** -0.5)   # 96**-0.5 (module bug kept faithful)

FB = mybir.dt.bfloat16
F32 = mybir.dt.float32

QS = [128, 128, 128, 128, 65]          # q-subtiles of 577
PCH = [(0, 512), (512, 65)]            # p-chunks of 577
VCH = [(0, 512), (512, 256)]           # V projection chunks of 768


def build_bass(reps=1, in_q="mix", out_q="sp", shared_pools=False,
               bias_eng="dve", vcopy_eng="dve"):
    nc = bacc.Bacc("TRN2", target_bir_lowering=False, debug=False,
                   num_devices=NCORES)

    x_t = nc.dram_tensor("x_t", [D, T], FB, kind="ExternalInput").ap()
    w_qkv_t = nc.dram_tensor("w_qkv_t", [D, 3 * D], FB, kind="ExternalInput").ap()
    w_out_t = nc.dram_tensor("w_out_t", [D, D], FB, kind="ExternalInput").ap()
    b_qk = nc.dram_tensor("b_qk", [P, 12], F32, kind="ExternalInput").ap()
    b_out = nc.dram_tensor("b_out", [P, DT], F32, kind="ExternalInput").ap()
    out_d = nc.dram_tensor("out", [D, T], FB, kind="ExternalOutput").ap()
    out_v = out_d.rearrange("(o p) t -> p o t", p=P)
    xv = x_t.rearrange("(o p) t -> p o t", p=P)
    wv = w_qkv_t.rearrange("(o p) e -> p o e", p=P)

    with tile.TileContext(nc) as tc:
      def emit_rep(singles, btp, atpool, nrm, ypool, pst, pg):
            # per-k input tiles so matmuls start after the first k arrives;
            # x DMA split per batch so batch-0 projections gate only on the
            # first quarter of each k-tile
            # biases on the ACT DMA queue (tiny; gate the DVE bias-adds of
            # every proj unit) so they don't delay the first weight DMA on SP
            bqk = singles.tile([P, 12], F32, tag="bqk")
            nc.scalar.dma_start(bqk[:], b_qk)
            bo = singles.tile([P, DT], F32, tag="bo")
            nc.scalar.dma_start(bo[:], b_out)
            # inputs optionally split across both HWDGE queues (SP + ACT) so
            # the two rings stream in parallel; q/k halves + batch-0 x first
            q2 = nc.scalar if in_q == "mix" else nc.sync
            xt, wqkv = [], []
            for k in range(DT):
                wk = singles.tile([P, 3 * D], FB, tag=f"wq{k}", name=f"wq{k}")
                nc.sync.dma_start(wk[:, D:3 * D], wv[:, k, D:3 * D])
                q2.dma_start(wk[:, 0:D], wv[:, k, 0:D])
                wqkv.append(wk)
                xk = singles.tile([P, T], FB, tag=f"xt{k}", name=f"xt{k}")
                q2.dma_start(xk[:, 0:PL], xv[:, k, 0:PL])
                xt.append(xk)
            # out-proj weights: one plain 2D DMA per 128-row block (a single
            # 3D-AP DMA here hits the slow SWDGE descriptor path and parks
            # the SP sequencer for ~50us)
            wo = singles.tile([P, DT, D], FB, tag="wo")
            wov = w_out_t.rearrange("(o p) e -> o p e", p=P)
            for o in range(DT):
                (nc.sync if o % 2 == 0 else q2).dma_start(
                    wo[:, o, :], wov[o])
            for k in range(DT):
                for b in range(1, NB):
                    (nc.sync if (k + b) % 2 == 0 else q2).dma_start(
                        xt[k][:, b * PL:(b + 1) * PL],
                        xv[:, k, b * PL:(b + 1) * PL])

            qkt, vbuf, ot, at = {}, {}, {}, {}

            def emit_proj_j(b, j):
                # q/k projection for one 128-feature block -> qkT[(b,j)]
                ps = pg.tile([P, D], F32, tag="prj", name="psqk")
                for k in range(DT):
                    for (c0, cw) in PCH:
                        nc.tensor.matmul(
                            ps[:, c0:c0 + cw],
                            lhsT=wqkv[k][:, D + j * P: D + (j + 1) * P],
                            rhs=xt[k][:, b * PL + c0: b * PL + c0 + cw],
                            start=(k == 0), stop=(k == DT - 1),
                            skip_group_check=True)
                qt_tile = btp.tile([P, PL], FB, tag=f"qkt{j}", name=f"qkt{j}")
                beng = nc.gpsimd if bias_eng == "pool" else nc.vector
                beng.tensor_scalar_add(qt_tile[:], ps[:, 0:PL],
                                       bqk[:, j:j + 1])
                qkt[(b, j)] = qt_tile

            def emit_projv(b, tt):
                # V projection for one 128-token block -> vbuf[(b,tt)]
                rows = QS[tt]
                ps = pg.tile([P, D], F32, tag="prj", name="psv")
                for k in range(DT):
                    for (c0, cw) in VCH:
                        nc.tensor.matmul(
                            ps[:rows, c0:c0 + cw],
                            lhsT=xt[k][:, b * PL + tt * P: b * PL + tt * P + rows],
                            rhs=wqkv[k][:, c0:c0 + cw],
                            start=(k == 0), stop=(k == DT - 1),
                            skip_group_check=True)
                vt = btp.tile([P, H, S + 1], FB, tag=f"v{tt}", name=f"v{tt}")
                nc.gpsimd.memset(vt[:, :, S:S + 1], 1.0)
                if vcopy_eng == "act":
                    nc.scalar.copy(
                        vt[:rows, :, 0:S],
                        ps[:rows].rearrange("p (h s) -> p h s", h=H))
                else:
                    nc.vector.tensor_copy(
                        vt[:rows, :, 0:S],
                        ps[:rows].rearrange("p (h s) -> p h s", h=H))
                vbuf[(b, tt)] = vt

            def emit_st(b, j2, qt):
                # scores + exp for one 128-key-token block of a head pair
                rows = QS[qt]
                qs_ = qkt[(b, j2)]
                ks_ = qkt[(b, 6 + j2)]
                if qt == 0:
                    at[(b, j2, 0)] = atpool.tile([P, 5, PL], FB, tag="at",
                                                 name="at0")
                    at[(b, j2, 1)] = atpool.tile([P, 5, PL], FB, tag="at",
                                                 name="at1")
                st0 = pst.tile([P, PL], F32, tag="st", name="st0")
                st1 = pst.tile([P, PL], F32, tag="st", name="st1")
                for (c0, cw) in PCH:
                    nc.tensor.matmul(
                        st0[:rows, c0:c0 + cw],
                        lhsT=ks_[0:64, qt * P: qt * P + rows],
                        rhs=qs_[0:64, c0:c0 + cw],
                        start=True, stop=True,
                        skip_group_check=True)
                    nc.tensor.matmul(
                        st1[:rows, c0:c0 + cw],
                        lhsT=ks_[64:128, qt * P: qt * P + rows],
                        rhs=qs_[64:128, c0:c0 + cw],
                        start=True, stop=True,
                        skip_group_check=True)
                nc.scalar.activation(
                    at[(b, j2, 0)][:rows, qt, :], st0[:rows, :],
                    mybir.ActivationFunctionType.Exp, scale=SCALE)
                nc.scalar.activation(
                    at[(b, j2, 1)][:rows, qt, :], st1[:rows, :],
                    mybir.ActivationFunctionType.Exp, scale=SCALE)

            avs_t, recb_t = {}, {}

            def emit_av_front(b, h):
                # attention-weighted V matmuls + fast PSUM drain (copy to
                # SBUF) + reciprocal; the normalize-multiply is deferred one
                # slot (emit_av_norm) so the PSUM buffer frees after just the
                # copy instead of the whole recip->broadcast->mul chain
                j2, half = h // 2, h % 2
                if h == 0:
                    for j in range(DT):
                        ot[(b, j)] = btp.tile([P, PL], FB, tag=f"ot{j}",
                                              name=f"ot{j}")
                ath = at[(b, j2, half)]
                av = pg.tile([P, D], F32, tag="prj", name="av")
                for qt in range(5):
                    rows = QS[qt]
                    for (c0, cw) in PCH:
                        nc.tensor.matmul(
                            av[0:S + 1, c0:c0 + cw],
                            lhsT=vbuf[(b, qt)][:rows, h, :],
                            rhs=ath[:rows, qt, c0:c0 + cw],
                            start=(qt == 0), stop=(qt == 4),
                            skip_group_check=True)
                # bf16 drain copy: 16-bit gets 2x DVE throughput; bf16
                # denominators cost ~0.4% rel err, well inside budget
                avs = nrm.tile([S + 1, PL], FB, tag="avs", name="avs")
                nc.vector.tensor_copy(avs[:], av[0:S + 1, 0:PL])
                rec = nrm.tile([1, PL], F32, tag="rec", name="rec")
                nc.vector.reciprocal(rec[:], avs[S:S + 1, :])
                recb = nrm.tile([64, PL], F32, tag="recb", name="recb")
                nc.gpsimd.partition_broadcast(recb[:], rec[:])
                avs_t[(b, h)] = avs
                recb_t[(b, h)] = recb

            def emit_av_norm(b, h):
                j2, hp = h // 2, (h % 2) * 64
                avs = avs_t.pop((b, h))
                recb = recb_t.pop((b, h))
                if hp == 0:
                    nc.vector.tensor_mul(
                        out=ot[(b, j2)][0:S, :], in0=avs[0:S, :], in1=recb[:])
                else:
                    tmp = nrm.tile([64, PL], FB, tag="tmp", name="tmp")
                    nc.vector.tensor_mul(out=tmp[:], in0=avs[0:S, :],
                                         in1=recb[:])
                    # gpsimd SWDGE queue: keeps this partition-shift copy off
                    # the SP queue where it would sit behind output DMAs
                    nc.gpsimd.dma_start(ot[(b, j2)][64:128, :], tmp[:])

            def emit_out(b, m):
                # output projection for one 128-feature block + store
                ps = pg.tile([P, D], F32, tag="prj", name="psy")
                for k in range(DT):
                    for (c0, cw) in PCH:
                        nc.tensor.matmul(
                            ps[:, c0:c0 + cw],
                            lhsT=wo[:, k, m * P:(m + 1) * P],
                            rhs=ot[(b, k)][:, c0:c0 + cw],
                            start=(k == 0), stop=(k == DT - 1),
                            skip_group_check=True)
                ysb = ypool.tile([P, PL], FB, tag="ysb", name="ysb")
                beng = nc.gpsimd if bias_eng == "pool" else nc.vector
                beng.tensor_scalar_add(ysb[:], ps[:, 0:PL], bo[:, m:m + 1])
                oq = nc.scalar if (out_q == "mix" and m % 2 == 1) else nc.sync
                oq.dma_start(out_v[:, m, b * PL:(b + 1) * PL], ysb[:])

            # ---- software-pipelined emission schedule ----
            # Each iteration (b, j2) emits its own batch's NEXT j2-pair of
            # q/k projections, so every batch (including the last) carries
            # its projection filler; batch b+1's first pair rides iter
            # (b, 5). Prologue covers only batch 0's first pair + V.
            emit_proj_j(0, 0)
            emit_proj_j(0, 6)
            for tt in range(5):
                emit_projv(0, tt)

            def proj_filler(b, j2, which):
                # which=0 -> first PROJ slot, which=1 -> second
                if j2 < 5:
                    j = (j2 + 1) if which == 0 else (j2 + 7)
                    emit_proj_j(b, j)
                elif b + 1 < NB:
                    emit_proj_j(b + 1, 0 if which == 0 else 6)

            for b in range(NB):
                for j2 in range(6):
                    # previous head pair to normalize: (b, j2-1), wrapping to
                    # (b-1, 5) at j2=0
                    pb, pj2 = (b, j2 - 1) if j2 > 0 else (b - 1, 5)
                    emit_st(b, j2, 0)
                    # PROJV early so its ACT V-copy's psum wait never
                    # head-of-line blocks the exp stream behind it
                    if b + 1 < NB and j2 < 5:
                        emit_projv(b + 1, j2)
                    emit_st(b, j2, 1)
                    proj_filler(b, j2, 0)
                    emit_st(b, j2, 2)
                    if pb >= 0:
                        emit_av_front(pb, 2 * pj2)
                    emit_st(b, j2, 3)
                    if pb >= 0:
                        emit_av_front(pb, 2 * pj2 + 1)
                        emit_av_norm(pb, 2 * pj2)
                    emit_st(b, j2, 4)
                    if pb >= 0:
                        emit_av_norm(pb, 2 * pj2 + 1)
                    if b > 0:
                        emit_out(b - 1, j2)
                    proj_filler(b, j2, 1)
            # epilogue: last head pair + last batch's output projection
            emit_av_front(NB - 1, 10)
            emit_av_front(NB - 1, 11)
            emit_av_norm(NB - 1, 10)
            emit_av_norm(NB - 1, 11)
            for m in range(DT):
                emit_out(NB - 1, m)

      def open_pools():
        return (tc.tile_pool(name="singles", bufs=1),
                tc.tile_pool(name="bt", bufs=2),
                tc.tile_pool(name="atp", bufs=6),
                tc.tile_pool(name="nrm", bufs=6),
                tc.tile_pool(name="yout", bufs=4),
                tc.tile_pool(name="pst", bufs=2, space="PSUM"),
                tc.tile_pool(name="pg", bufs=2, space="PSUM"))

      import contextlib
      if shared_pools:
          # one pool generation across reps: no inter-rep drain barrier
          with contextlib.ExitStack() as st:
              pools = [st.enter_context(p) for p in open_pools()]
              for _rep in range(reps):
                  emit_rep(*pools)
      else:
          for _rep in range(reps):
              with contextlib.ExitStack() as st:
                  pools = [st.enter_context(p) for p in open_pools()]
                  emit_rep(*pools)

    nc.compile()
    return nc


_NC = None


def _get_nc():
    global _NC
    if _NC is None:
        _NC = build_bass()
    return _NC


def make_in_maps(x, qkv_w, qkv_b, out_w, out_b):
    """Host-side shard + layout prep. Returns per-core input dicts."""
    bf16 = ml_dtypes.bfloat16
    x = np.asarray(x, dtype=np.float32)
    qkv_w = np.asarray(qkv_w, dtype=np.float32)
    qkv_b = np.asarray(qkv_b, dtype=np.float32)
    out_w = np.asarray(out_w, dtype=np.float32)
    out_b = np.asarray(out_b, dtype=np.float32)

    w_qkv_t = np.ascontiguousarray(qkv_w.T).astype(bf16)          # [768, 2304]
    w_out_t = np.ascontiguousarray(out_w.T).astype(bf16)          # [768, 768]
    b_qk = np.ascontiguousarray(qkv_b[D:3 * D].reshape(12, P).T)  # [128, 12]
    # v-bias passes linearly through the output projection (softmax rows sum
    # to 1): fold it into an effective output bias.
    b_eff = out_b + out_w @ qkv_b[0:D]
    b_out = np.ascontiguousarray(b_eff.reshape(DT, P).T)          # [128, 6]

    in_maps = []
    for c in range(NCORES):
        xc = x[c * NB:(c + 1) * NB].reshape(T, D)                 # [2308, 768]
        x_t = np.ascontiguousarray(xc.T).astype(bf16)             # [768, 2308]
        in_maps.append({
            "x_t": x_t,
            "w_qkv_t": w_qkv_t,
            "w_out_t": w_out_t,
            "b_qk": b_qk.astype(np.float32),
            "b_out": b_out.astype(np.float32),
        })
    return in_maps


def assemble_output(results):
    """Per-core 'out' [768, 2308] bf16 -> full [32, 577, 768] f32."""
    y = np.empty((B, PL, D), dtype=np.float32)
    for c in range(NCORES):
        yt = results[c]["out"].astype(np.float32)                 # [768, 2308]
        y[c * NB:(c + 1) * NB] = yt.T.reshape(NB, PL, D)
    return y


def run(x, qkv_w, qkv_b, out_w, out_b, trace=False):
    nc = _get_nc()
    in_maps = make_in_maps(x, qkv_w, qkv_b, out_w, out_b)
    res = run_bass_kernel_spmd(nc, in_maps, core_ids=list(range(NCORES)),
                               trace=trace)
    return assemble_output(res.results), res


def kernel(x, qkv_w, qkv_b, out_w, out_b):
    y, _ = run(x, qkv_w, qkv_b, out_w, out_b)
    return y
